# revision 5
# baseline (speedup 1.0000x reference)
"""Trainium2 Bass kernel: multi-head attention (B=4, N=2048, C=768, H=12, D=64).

Sharding over 8 NeuronCores: core c -> (batch b = c//2, head-group g = c%2).
Each head-group is 6 heads (384 channels). Per core:
  - qkv projection for its 6 heads (f32r matmuls, full-rate fp32)
  - per-head attention with transposed score layout scoreT[k, q]:
      * no max-subtraction (scores ~ N(0,1), exp is safe in fp32)
      * softmax denominator comes free from a ones-column appended to V
  - output projection against the head-group's slice of proj_w -> partial out
  - ReduceScatter(add) over the pair sharing a batch -> each core holds half
    the rows of out[b]; host concatenates.

All device tensors are pre-transposed on the host so the kernel needs no
on-chip transposes:
  xT   [768, 2048]  = x[b].T
  qwT/kwT/vwT [768, 384] = per-group qkv weight slices, transposed
  pwT  [384, 768]   = proj_w[:, group_cols].T
"""

import numpy as np

B, N, C = 4, 2048, 768
H, D = 12, 64
NH = 6            # heads per core
HD = NH * D       # 384 channels per core
CT = C // 128     # 6 contraction tiles
NT = N // 128     # 16 n tiles of 128
NCOL = N // 512   # 4 n columns of 512
HDT = HD // 128   # 3 head-dim tiles (2 heads each)
VG = D + 1        # 65: v columns + ones column per head

_CACHE = {}


def build_nc():
    import concourse.bass as bass
    import concourse.mybir as mybir
    import concourse.tile as tile
    from concourse import bacc

    F32 = mybir.dt.float32
    F32R = mybir.dt.float32r
    BF16 = mybir.dt.bfloat16

    nc = bacc.Bacc("TRN2", target_bir_lowering=False, debug=False, num_devices=8)

    xT_d = nc.declare_dram_parameter("xT", [C, N], F32R, isOutput=False)
    qwT_d = nc.declare_dram_parameter("qwT", [C, HD], F32R, isOutput=False)
    kwT_d = nc.declare_dram_parameter("kwT", [C, HD], F32R, isOutput=False)
    vwT_d = nc.declare_dram_parameter("vwT", [C, HD], F32R, isOutput=False)
    pwT_d = nc.declare_dram_parameter("pwT", [HD, C], BF16, isOutput=False)
    ones_d = nc.declare_dram_parameter("ones", [128, 64], F32, isOutput=False)
    out_d = nc.declare_dram_parameter("out", [N // 2, C], F32, isOutput=True)

    scale = 1.0 / np.sqrt(D)

    with tile.TileContext(nc) as tc:
        with (
            tc.tile_pool(name="w", bufs=1) as wpool,
            tc.tile_pool(name="big", bufs=1) as big,
            tc.tile_pool(name="qk", bufs=1) as qkpool,
            tc.tile_pool(name="exp", bufs=8) as epool,
            tc.tile_pool(name="sm", bufs=2) as smpool,
            tc.tile_pool(name="outs", bufs=4) as opool,
            tc.tile_pool(name="ps", bufs=8, space="PSUM") as ps,
            tc.tile_pool(name="dram", bufs=1, space="DRAM") as dram,
        ):
            # ---- load weights & constants ----
            qw_s = wpool.tile([128, CT, HD], F32R, tag="qw")
            kw_s = wpool.tile([128, CT, HD], F32R, tag="kw")
            vw_s = wpool.tile([128, CT, HD], F32R, tag="vw")
            pw_s = wpool.tile([64, NH, C], BF16, tag="pw")
            ones_s = wpool.tile([128, 64], F32, tag="ones")
            nc.sync.dma_start(out=qw_s[:], in_=qwT_d.rearrange("(t p) f -> p t f", p=128))
            nc.sync.dma_start(out=kw_s[:], in_=kwT_d.rearrange("(t p) f -> p t f", p=128))
            nc.sync.dma_start(out=vw_s[:], in_=vwT_d.rearrange("(t p) f -> p t f", p=128))
            nc.sync.dma_start(out=pw_s[:], in_=pwT_d.rearrange("(h p) f -> p h f", p=64))
            nc.sync.dma_start(out=ones_s[:], in_=ones_d[:, :])

            xt_s = big.tile([128, CT, N], F32R, tag="xt")
            nc.sync.dma_start(out=xt_s[:], in_=xT_d.rearrange("(t p) n -> p t n", p=128))

            # persistent SBUF tensors
            qT = [qkpool.tile([128, N], F32R, tag=f"qT{t}", name=f"qT{t}") for t in range(HDT)]
            kT = [qkpool.tile([128, N], F32R, tag=f"kT{t}", name=f"kT{t}") for t in range(HDT)]
            v_s = big.tile([128, NT, NH * VG], BF16, tag="v")
            ctx = [qkpool.tile([64, N], BF16, tag=f"ctx{h}", name=f"ctx{h}") for h in range(NH)]

            # ones columns of v (column VG-1 of each per-head group)
            v4 = v_s.rearrange("p n (h g) -> p n h g", h=NH)
            nc.vector.memset(v4[:, :, :, D : D + 1], 1.0)

            # ---- phase A: qkv projections ----
            # qT/kT: [hd, n] tiles; v: natural [n, hd]
            for hdt in range(HDT):
                for ncol in range(NCOL):
                    nsl = slice(ncol * 512, ncol * 512 + 512)
                    pq = ps.tile([128, 512], F32, tag="ps")
                    for ct in range(CT):
                        nc.tensor.matmul(
                            pq[:, :],
                            qw_s[:, ct, hdt * 128 : hdt * 128 + 128],
                            xt_s[:, ct, nsl],
                            start=(ct == 0), stop=(ct == CT - 1),
                        )
                    nc.vector.tensor_copy(out=qT[hdt][:, nsl], in_=pq[:, :])
                    pk = ps.tile([128, 512], F32, tag="ps")
                    for ct in range(CT):
                        nc.tensor.matmul(
                            pk[:, :],
                            kw_s[:, ct, hdt * 128 : hdt * 128 + 128],
                            xt_s[:, ct, nsl],
                            start=(ct == 0), stop=(ct == CT - 1),
                        )
                    nc.vector.tensor_copy(out=kT[hdt][:, nsl], in_=pk[:, :])

            for nt in range(NT):
                pv = ps.tile([128, 512], F32, tag="ps")
                for ct in range(CT):
                    nc.tensor.matmul(
                        pv[:, :HD],
                        xt_s[:, ct, nt * 128 : nt * 128 + 128],
                        vw_s[:, ct, :],
                        start=(ct == 0), stop=(ct == CT - 1),
                    )
                nc.vector.tensor_copy(
                    out=v4[:, nt, :, 0:D],
                    in_=pv[:, :HD].rearrange("p (h d) -> p h d", h=NH),
                )

            # ---- phase B: attention (ncol outer so proj can follow per ncol) ----
            out_bounce = dram.tile([N, C], F32)
            rs_out = dram.tile([N // 2, C], F32)

            for ncol in range(NCOL):
                nsl = slice(ncol * 512, ncol * 512 + 512)
                for hp in range(HDT):  # head pairs share a qT/kT tile
                    pctx = [ps.tile([128, 512], F32, tag="ps", name=f"pctx{_i}") for _i in range(2)]
                    for kt in range(NT):
                        ksl = slice(kt * 128, kt * 128 + 128)
                        for sub in range(2):  # head hp*2+sub at d-partitions 64*sub..
                            h = hp * 2 + sub
                            p0 = 64 * sub
                            pscore = ps.tile([128, 512], F32, tag="ps")
                            nc.tensor.matmul(
                                pscore[:, :],
                                kT[hp][p0 : p0 + 64, ksl],
                                qT[hp][p0 : p0 + 64, nsl],
                                start=True, stop=True,
                            )
                            et = epool.tile([128, 512], BF16, tag="exp")
                            nc.scalar.activation(
                                out=et[:, :], in_=pscore[:, :],
                                func=mybir.ActivationFunctionType.Exp,
                                scale=float(scale),
                            )
                            nc.tensor.matmul(
                                pctx[sub][0:VG, :],
                                v_s[:, kt, h * VG : h * VG + VG],
                                et[:, :],
                                start=(kt == 0), stop=(kt == NT - 1),
                            )
                    for sub in range(2):
                        h = hp * 2 + sub
                        # normalize: recip of sum row, broadcast via outer product
                        rc = smpool.tile([65, 512], F32, tag="rc")
                        nc.vector.reciprocal(out=rc[64:65, :], in_=pctx[sub][64:65, :])
                        pbc = ps.tile([128, 512], F32, tag="ps")
                        nc.tensor.matmul(
                            pbc[0:64, :], ones_s[64:65, 0:64], rc[64:65, :],
                            start=True, stop=True,
                        )
                        bc = smpool.tile([64, 512], F32, tag="bc")
                        nc.vector.tensor_copy(out=bc[:, :], in_=pbc[0:64, :])
                        nc.vector.tensor_tensor(
                            ctx[h][:, nsl], pctx[sub][0:64, :], bc[:, :],
                            mybir.AluOpType.mult,
                        )

                # ---- phase C: proj for the 4 n-tiles of this ncol ----
                for nti in range(4):
                    nt = ncol * 4 + nti
                    for ocol in range(2):
                        po = ps.tile([128, 512], F32, tag="ps")
                        for h in range(NH):
                            nc.tensor.matmul(
                                po[:, 0:384],
                                ctx[h][:, nt * 128 : nt * 128 + 128],
                                pw_s[:, h, ocol * 384 : ocol * 384 + 384],
                                start=(h == 0), stop=(h == NH - 1),
                            )
                        ot = opool.tile([128, 384], F32, tag="ot")
                        nc.vector.tensor_copy(out=ot[:, :], in_=po[:, 0:384])
                        nc.sync.dma_start(
                            out=out_bounce[nt * 128 : nt * 128 + 128, ocol * 384 : ocol * 384 + 384],
                            in_=ot[:, :],
                        )

            # ---- ReduceScatter over the batch pair ----
            nc.gpsimd.collective_compute(
                "ReduceScatter",
                mybir.AluOpType.add,
                replica_groups=[[0, 1], [2, 3], [4, 5], [6, 7]],
                ins=[out_bounce.opt()],
                outs=[rs_out.opt()],
            )
            nc.sync.dma_start(out=out_d[:, :], in_=rs_out[:])

    nc.compile()
    return nc


def _prep_inputs(x, qkv_w, proj_w):
    """Build per-core input maps (host-side slicing/transposes only)."""
    ones = np.ones((128, 64), dtype=np.float32)
    try:
        import ml_dtypes
        bf16 = ml_dtypes.bfloat16
    except ImportError:
        bf16 = None
    in_maps = []
    for c in range(8):
        b, g = c // 2, c % 2
        sl = slice(g * HD, g * HD + HD)
        pwT = np.ascontiguousarray(proj_w[:, sl].T)
        pwT = pwT.astype(bf16) if bf16 is not None else pwT
        in_maps.append({
            "xT": np.ascontiguousarray(x[b].T),
            "qwT": np.ascontiguousarray(qkv_w[0 * C + g * HD : 0 * C + g * HD + HD, :].T),
            "kwT": np.ascontiguousarray(qkv_w[1 * C + g * HD : 1 * C + g * HD + HD, :].T),
            "vwT": np.ascontiguousarray(qkv_w[2 * C + g * HD : 2 * C + g * HD + HD, :].T),
            "pwT": pwT,
            "ones": ones,
        })
    return in_maps


def kernel(x, qkv_w, proj_w):
    from concourse.bass_utils import run_bass_kernel_spmd

    x = np.asarray(x, dtype=np.float32)
    qkv_w = np.asarray(qkv_w, dtype=np.float32)
    proj_w = np.asarray(proj_w, dtype=np.float32)

    if "nc" not in _CACHE:
        _CACHE["nc"] = build_nc()
    nc = _CACHE["nc"]

    in_maps = _prep_inputs(x, qkv_w, proj_w)
    res = run_bass_kernel_spmd(nc, in_maps, core_ids=list(range(8)))

    out = np.empty((B, N, C), dtype=np.float32)
    for b in range(B):
        out[b, : N // 2] = res.results[2 * b]["out"]
        out[b, N // 2 :] = res.results[2 * b + 1]["out"]
    return out


# revision 7
# speedup vs baseline: 1.1639x; 1.1639x over previous
"""Trainium2 Bass kernel: multi-head attention (B=4, N=2048, C=768, H=12, D=64).

Sharding over 8 NeuronCores: core c -> (batch b = c//2, head-group g = c%2).
Each head-group is 6 heads (384 channels). Per core:
  - qkv projection for its 6 heads (f32r matmuls, full-rate fp32)
  - per-head attention with transposed score layout scoreT[k, q]:
      * no max-subtraction (scores ~ N(0,1), exp is safe in fp32)
      * softmax denominator comes free from a ones-column appended to V
  - output projection against the head-group's slice of proj_w -> partial out
  - ReduceScatter(add) over the pair sharing a batch -> each core holds half
    the rows of out[b]; host concatenates.

All device tensors are pre-transposed on the host so the kernel needs no
on-chip transposes:
  xT   [768, 2048]  = x[b].T
  qwT/kwT/vwT [768, 384] = per-group qkv weight slices, transposed
  pwT  [384, 768]   = proj_w[:, group_cols].T

Perf notes (v2): q/k stored bf16 so score matmuls get FWL + background
weight loads and count toward the PE clock-gate's activity monitor; the two
heads of a pair run in distinct 64-row groups of the array (concurrent);
V per-head groups padded to 128 columns ([v | ones | zeros]) for FWL;
exp processed in 1024-wide blocks to amortize ScalarE per-op overhead.
"""

import numpy as np

B, N, C = 4, 2048, 768
H, D = 12, 64
NH = 6            # heads per core
HD = NH * D       # 384 channels per core
CT = C // 128     # 6 contraction tiles
NT = N // 128     # 16 k tiles of 128
NBLK = N // 1024  # 2 q blocks of 1024
HDT = HD // 128   # 3 head-dim tiles (2 heads each)
VP = 128          # padded v group: [v(64) | ones | zeros(63)]

_CACHE = {}


def build_nc():
    import concourse.bass as bass
    import concourse.mybir as mybir
    import concourse.tile as tile
    from concourse import bacc

    F32 = mybir.dt.float32
    F32R = mybir.dt.float32r
    BF16 = mybir.dt.bfloat16

    nc = bacc.Bacc("TRN2", target_bir_lowering=False, debug=False, num_devices=8)

    xT_d = nc.declare_dram_parameter("xT", [C, N], F32R, isOutput=False)
    qwT_d = nc.declare_dram_parameter("qwT", [C, HD], F32R, isOutput=False)
    kwT_d = nc.declare_dram_parameter("kwT", [C, HD], F32R, isOutput=False)
    vwT_d = nc.declare_dram_parameter("vwT", [C, HD], F32R, isOutput=False)
    pwT_d = nc.declare_dram_parameter("pwT", [HD, C], BF16, isOutput=False)
    ones_d = nc.declare_dram_parameter("ones", [128, 64], BF16, isOutput=False)
    out_d = nc.declare_dram_parameter("out", [N // 2, C], F32, isOutput=True)

    scale = 1.0 / np.sqrt(D)

    with tile.TileContext(nc) as tc:
        with (
            tc.tile_pool(name="w", bufs=1) as wpool,
            tc.tile_pool(name="big", bufs=1) as big,
            tc.tile_pool(name="qk", bufs=1) as qkpool,
            tc.tile_pool(name="exp", bufs=6) as epool,
            tc.tile_pool(name="sm", bufs=2) as smpool,
            tc.tile_pool(name="outs", bufs=4) as opool,
            tc.tile_pool(name="ps", bufs=4, space="PSUM") as ps,
            tc.tile_pool(name="dram", bufs=1, space="DRAM") as dram,
        ):
            # ---- load weights & constants ----
            qw_s = wpool.tile([128, CT, HD], F32R, tag="qw")
            kw_s = wpool.tile([128, CT, HD], F32R, tag="kw")
            vw_s = wpool.tile([128, CT, HD], F32R, tag="vw")
            pw_s = wpool.tile([64, NH, C], BF16, tag="pw")
            ones_s = wpool.tile([128, 64], BF16, tag="ones")
            nc.sync.dma_start(out=qw_s[:], in_=qwT_d.rearrange("(t p) f -> p t f", p=128))
            nc.sync.dma_start(out=kw_s[:], in_=kwT_d.rearrange("(t p) f -> p t f", p=128))
            nc.sync.dma_start(out=vw_s[:], in_=vwT_d.rearrange("(t p) f -> p t f", p=128))
            nc.sync.dma_start(out=pw_s[:], in_=pwT_d.rearrange("(h p) f -> p h f", p=64))
            nc.sync.dma_start(out=ones_s[:], in_=ones_d[:, :])

            xt_s = big.tile([128, CT, N], F32R, tag="xt")
            nc.sync.dma_start(out=xt_s[:], in_=xT_d.rearrange("(t p) n -> p t n", p=128))

            # persistent SBUF tensors
            qT = [qkpool.tile([128, N], BF16, tag=f"qT{t}", name=f"qT{t}") for t in range(HDT)]
            kT = [qkpool.tile([128, N], BF16, tag=f"kT{t}", name=f"kT{t}") for t in range(HDT)]
            v_s = big.tile([128, NT, NH * VP], BF16, tag="v")
            ctx = [qkpool.tile([64, N], BF16, tag=f"ctx{h}", name=f"ctx{h}") for h in range(NH)]

            # v group layout per head: [v(64) | ones(1) | zeros(63)]
            v4 = v_s.rearrange("p n (h g) -> p n h g", h=NH)
            nc.vector.memset(v4[:, :, :, D:], 0.0)
            nc.vector.memset(v4[:, :, :, D : D + 1], 1.0)

            # ---- phase A: qkv projections ----
            for hdt in range(HDT):
                for ncol in range(NCOL4):
                    nsl = slice(ncol * 512, ncol * 512 + 512)
                    pq = ps.tile([128, 1024], F32, tag="ps")
                    for ct in range(CT):
                        nc.tensor.matmul(
                            pq[:, 0:512],
                            qw_s[:, ct, hdt * 128 : hdt * 128 + 128],
                            xt_s[:, ct, nsl],
                            start=(ct == 0), stop=(ct == CT - 1),
                        )
                    for ct in range(CT):
                        nc.tensor.matmul(
                            pq[:, 512:1024],
                            kw_s[:, ct, hdt * 128 : hdt * 128 + 128],
                            xt_s[:, ct, nsl],
                            start=(ct == 0), stop=(ct == CT - 1),
                        )
                    nc.vector.tensor_copy(out=qT[hdt][:, nsl], in_=pq[:, 0:512])
                    nc.vector.tensor_copy(out=kT[hdt][:, nsl], in_=pq[:, 512:1024])

            for nt in range(NT):
                pv = ps.tile([128, 1024], F32, tag="ps")
                for ct in range(CT):
                    nc.tensor.matmul(
                        pv[:, :HD],
                        xt_s[:, ct, nt * 128 : nt * 128 + 128],
                        vw_s[:, ct, :],
                        start=(ct == 0), stop=(ct == CT - 1),
                    )
                nc.vector.tensor_copy(
                    out=v4[:, nt, :, 0:D],
                    in_=pv[:, :HD].rearrange("p (h d) -> p h d", h=NH),
                )

            # ---- phase B: attention (q-block outer; proj follows per block) ----
            out_bounce = dram.tile([N, C], F32)
            rs_out = dram.tile([N // 2, C], F32)

            for blk in range(NBLK):
                q0 = blk * 1024
                bsl = slice(q0, q0 + 1024)
                for hp in range(HDT):  # head pairs share a qT/kT tile
                    pctx = [ps.tile([128, 1024], F32, tag="ps", name=f"pctx{_i}") for _i in range(2)]
                    for kt in range(NT):
                        ksl = slice(kt * 128, kt * 128 + 128)
                        pscore = [None, None]
                        for sub in range(2):  # heads in distinct 64-row array groups
                            p0 = 64 * sub
                            psc = ps.tile([128, 1024], F32, tag="ps", name=f"psc{sub}")
                            for qh in range(2):
                                nc.tensor.matmul(
                                    psc[:, qh * 512 : qh * 512 + 512],
                                    kT[hp][p0 : p0 + 64, ksl],
                                    qT[hp][p0 : p0 + 64, q0 + qh * 512 : q0 + qh * 512 + 512],
                                    start=True, stop=True,
                                )
                            pscore[sub] = psc
                        ets = [None, None]
                        for sub in range(2):
                            et = epool.tile([128, 1024], BF16, tag="exp", name=f"et{sub}")
                            nc.scalar.activation(
                                out=et[:, :], in_=pscore[sub][:, :],
                                func=mybir.ActivationFunctionType.Exp,
                                scale=float(scale),
                            )
                            ets[sub] = et
                        for sub in range(2):
                            h = hp * 2 + sub
                            for qh in range(2):
                                nc.tensor.matmul(
                                    pctx[sub][:, qh * 512 : qh * 512 + 512],
                                    v_s[:, kt, h * VP : h * VP + VP],
                                    ets[sub][:, qh * 512 : qh * 512 + 512],
                                    start=(kt == 0), stop=(kt == NT - 1),
                                )
                    for sub in range(2):
                        h = hp * 2 + sub
                        # normalize: 1/sum row (approx), broadcast via outer product
                        rc = smpool.tile([65, 1024], F32, tag="rc")
                        rcb = smpool.tile([65, 1024], BF16, tag="rcb")
                        nc.vector.reciprocal(out=rc[64:65, :], in_=pctx[sub][64:65, :])
                        nc.vector.tensor_copy(out=rcb[64:65, :], in_=rc[64:65, :])
                        pbc = ps.tile([128, 1024], F32, tag="ps")
                        for qh in range(2):
                            nc.tensor.matmul(
                                pbc[0:64, qh * 512 : qh * 512 + 512],
                                ones_s[64:65, 0:64],
                                rcb[64:65, qh * 512 : qh * 512 + 512],
                                start=True, stop=True,
                            )
                        bc = smpool.tile([64, 1024], F32, tag="bc")
                        nc.vector.tensor_copy(out=bc[:, :], in_=pbc[0:64, :])
                        nc.vector.tensor_tensor(
                            ctx[h][:, bsl], pctx[sub][0:64, :], bc[:, :],
                            mybir.AluOpType.mult,
                        )

                # ---- phase C: proj for the 8 n-tiles of this block ----
                for nti in range(8):
                    nt = blk * 8 + nti
                    po = ps.tile([128, 1024], F32, tag="ps")
                    for ocol in range(2):
                        for h in range(NH):
                            nc.tensor.matmul(
                                po[:, ocol * 512 : ocol * 512 + 384],
                                ctx[h][:, nt * 128 : nt * 128 + 128],
                                pw_s[:, h, ocol * 384 : ocol * 384 + 384],
                                start=(h == 0), stop=(h == NH - 1),
                            )
                    ot = opool.tile([128, C], F32, tag="ot")
                    nc.vector.tensor_copy(
                        out=ot[:, :].rearrange("p (o f) -> p o f", o=2),
                        in_=po[:, :].rearrange("p (o f) -> p o f", o=2)[:, :, 0:384],
                    )
                    nc.sync.dma_start(
                        out=out_bounce[nt * 128 : nt * 128 + 128, :],
                        in_=ot[:, :],
                    )

            # ---- ReduceScatter over the batch pair ----
            nc.gpsimd.collective_compute(
                "ReduceScatter",
                mybir.AluOpType.add,
                replica_groups=[[0, 1], [2, 3], [4, 5], [6, 7]],
                ins=[out_bounce.opt()],
                outs=[rs_out.opt()],
            )
            nc.sync.dma_start(out=out_d[:, :], in_=rs_out[:])

    nc.compile()
    return nc


NCOL4 = N // 512  # 4 columns of 512 for the qkv phase


def _prep_inputs(x, qkv_w, proj_w):
    """Build per-core input maps (host-side slicing/transposes only)."""
    import ml_dtypes
    bf16 = ml_dtypes.bfloat16
    ones = np.ones((128, 64), dtype=bf16)
    in_maps = []
    for c in range(8):
        b, g = c // 2, c % 2
        sl = slice(g * HD, g * HD + HD)
        pwT = np.ascontiguousarray(proj_w[:, sl].T).astype(bf16)
        in_maps.append({
            "xT": np.ascontiguousarray(x[b].T),
            "qwT": np.ascontiguousarray(qkv_w[0 * C + g * HD : 0 * C + g * HD + HD, :].T),
            "kwT": np.ascontiguousarray(qkv_w[1 * C + g * HD : 1 * C + g * HD + HD, :].T),
            "vwT": np.ascontiguousarray(qkv_w[2 * C + g * HD : 2 * C + g * HD + HD, :].T),
            "pwT": pwT,
            "ones": ones,
        })
    return in_maps


def kernel(x, qkv_w, proj_w):
    from concourse.bass_utils import run_bass_kernel_spmd

    x = np.asarray(x, dtype=np.float32)
    qkv_w = np.asarray(qkv_w, dtype=np.float32)
    proj_w = np.asarray(proj_w, dtype=np.float32)

    if "nc" not in _CACHE:
        _CACHE["nc"] = build_nc()
    nc = _CACHE["nc"]

    in_maps = _prep_inputs(x, qkv_w, proj_w)
    res = run_bass_kernel_spmd(nc, in_maps, core_ids=list(range(8)))

    out = np.empty((B, N, C), dtype=np.float32)
    for b in range(B):
        out[b, : N // 2] = res.results[2 * b]["out"]
        out[b, N // 2 :] = res.results[2 * b + 1]["out"]
    return out


# revision 10
# speedup vs baseline: 1.1686x; 1.0040x over previous
"""Trainium2 Bass kernel: multi-head attention (B=4, N=2048, C=768, H=12, D=64).

Sharding over 8 NeuronCores: core c -> (batch b = c//2, head-group g = c%2).
Each head-group is 6 heads (384 channels). Per core:
  - qkv projection for its 6 heads (f32r matmuls, full-rate fp32)
  - per-head attention with transposed score layout scoreT[k, q]:
      * no max-subtraction (scores ~ N(0,1), exp is safe in fp32)
      * softmax denominator comes free from a ones-column appended to V
  - output projection against the head-group's slice of proj_w -> partial out
  - ReduceScatter(add) over the pair sharing a batch -> each core holds half
    the rows of out[b]; host concatenates.

All device tensors are pre-transposed on the host so the kernel needs no
on-chip transposes:
  xT   [768, 2048]  = x[b].T
  qwT/kwT/vwT [768, 384] = per-group qkv weight slices, transposed
  pwT  [384, 768]   = proj_w[:, group_cols].T

Perf notes (v2): q/k stored bf16 so score matmuls get FWL + background
weight loads and count toward the PE clock-gate's activity monitor; the two
heads of a pair run in distinct 64-row groups of the array (concurrent);
V per-head groups padded to 128 columns ([v | ones | zeros]) for FWL;
exp processed in 1024-wide blocks to amortize ScalarE per-op overhead.
"""

import numpy as np

B, N, C = 4, 2048, 768
H, D = 12, 64
NH = 6            # heads per core
HD = NH * D       # 384 channels per core
CT = C // 128     # 6 contraction tiles
NT = N // 128     # 16 k tiles of 128
NBLK = N // 1024  # 2 q blocks of 1024
HDT = HD // 128   # 3 head-dim tiles (2 heads each)
VP = 128          # padded v group: [v(64) | ones | zeros(63)]

_CACHE = {}


def build_nc():
    import concourse.bass as bass
    import concourse.mybir as mybir
    import concourse.tile as tile
    from concourse import bacc

    F32 = mybir.dt.float32
    F32R = mybir.dt.float32r
    BF16 = mybir.dt.bfloat16

    nc = bacc.Bacc("TRN2", target_bir_lowering=False, debug=False, num_devices=8)

    xT_d = nc.declare_dram_parameter("xT", [C, N], F32R, isOutput=False)
    qwT_d = nc.declare_dram_parameter("qwT", [C, HD], F32R, isOutput=False)
    kwT_d = nc.declare_dram_parameter("kwT", [C, HD], F32R, isOutput=False)
    vwT_d = nc.declare_dram_parameter("vwT", [C, HD], F32R, isOutput=False)
    pwT_d = nc.declare_dram_parameter("pwT", [HD, C], BF16, isOutput=False)
    ones_d = nc.declare_dram_parameter("ones", [128, 64], F32, isOutput=False)
    out_d = nc.declare_dram_parameter("out", [N // 2, C], F32, isOutput=True)

    scale = 1.0 / np.sqrt(D)

    with tile.TileContext(nc) as tc:
        with (
            tc.tile_pool(name="w", bufs=1) as wpool,
            tc.tile_pool(name="big", bufs=1) as big,
            tc.tile_pool(name="qk", bufs=1) as qkpool,
            tc.tile_pool(name="exp", bufs=6) as epool,
            tc.tile_pool(name="sm", bufs=2) as smpool,
            tc.tile_pool(name="outs", bufs=2) as opool,
            tc.tile_pool(name="ps", bufs=4, space="PSUM") as ps,
            tc.tile_pool(name="dram", bufs=1, space="DRAM") as dram,
        ):
            # ---- load weights & constants ----
            qw_s = wpool.tile([128, CT, HD], F32R, tag="qw")
            kw_s = wpool.tile([128, CT, HD], F32R, tag="kw")
            vw_s = wpool.tile([128, CT, HD], F32R, tag="vw")
            pw_s = wpool.tile([64, NH, C], BF16, tag="pw")
            ones_s = wpool.tile([128, 64], F32, tag="ones")
            nc.sync.dma_start(out=qw_s[:], in_=qwT_d.rearrange("(t p) f -> p t f", p=128))
            nc.sync.dma_start(out=kw_s[:], in_=kwT_d.rearrange("(t p) f -> p t f", p=128))
            nc.sync.dma_start(out=vw_s[:], in_=vwT_d.rearrange("(t p) f -> p t f", p=128))
            nc.sync.dma_start(out=pw_s[:], in_=pwT_d.rearrange("(h p) f -> p h f", p=64))
            nc.sync.dma_start(out=ones_s[:], in_=ones_d[:, :])

            xt_s = big.tile([128, CT, N], F32R, tag="xt")
            nc.sync.dma_start(out=xt_s[:], in_=xT_d.rearrange("(t p) n -> p t n", p=128))

            # persistent SBUF tensors
            qT = [qkpool.tile([128, N], BF16, tag=f"qT{t}", name=f"qT{t}") for t in range(HDT)]
            kT = [qkpool.tile([128, N], BF16, tag=f"kT{t}", name=f"kT{t}") for t in range(HDT)]
            v_s = big.tile([128, NT, NH * VP], BF16, tag="v")
            ctx = [qkpool.tile([64, N], BF16, tag=f"ctx{h}", name=f"ctx{h}") for h in range(NH)]

            # v group layout per head: [v(64) | ones(1) | zeros(63)]
            v4 = v_s.rearrange("p n (h g) -> p n h g", h=NH)
            nc.vector.memset(v4[:, :, :, D:], 0.0)
            nc.vector.memset(v4[:, :, :, D : D + 1], 1.0)

            # ---- phase A: qkv projections ----
            for hdt in range(HDT):
                for ncol in range(NCOL4):
                    nsl = slice(ncol * 512, ncol * 512 + 512)
                    pq = ps.tile([128, 1024], F32, tag="ps")
                    for ct in range(CT):
                        nc.tensor.matmul(
                            pq[:, 0:512],
                            qw_s[:, ct, hdt * 128 : hdt * 128 + 128],
                            xt_s[:, ct, nsl],
                            start=(ct == 0), stop=(ct == CT - 1),
                        )
                    for ct in range(CT):
                        nc.tensor.matmul(
                            pq[:, 512:1024],
                            kw_s[:, ct, hdt * 128 : hdt * 128 + 128],
                            xt_s[:, ct, nsl],
                            start=(ct == 0), stop=(ct == CT - 1),
                        )
                    nc.vector.tensor_copy(out=qT[hdt][:, nsl], in_=pq[:, 0:512])
                    nc.vector.tensor_copy(out=kT[hdt][:, nsl], in_=pq[:, 512:1024])

            for nt in range(NT):
                pv = ps.tile([128, 1024], F32, tag="ps")
                for ct in range(CT):
                    nc.tensor.matmul(
                        pv[:, :HD],
                        xt_s[:, ct, nt * 128 : nt * 128 + 128],
                        vw_s[:, ct, :],
                        start=(ct == 0), stop=(ct == CT - 1),
                    )
                nc.vector.tensor_copy(
                    out=v4[:, nt, :, 0:D],
                    in_=pv[:, :HD].rearrange("p (h d) -> p h d", h=NH),
                )

            # ---- phase B: attention (q-block outer; proj follows per block) ----
            out_bounce = dram.tile([N, C], F32)
            rs_out = dram.tile([N // 2, C], F32)

            for blk in range(NBLK):
                q0 = blk * 1024
                bsl = slice(q0, q0 + 1024)
                for hp in range(HDT):  # head pairs share a qT/kT tile
                    pctx = [ps.tile([128, 1024], F32, tag="ps", name=f"pctx{_i}") for _i in range(2)]
                    for kt in range(NT):
                        ksl = slice(kt * 128, kt * 128 + 128)
                        pscore = [
                            ps.tile([128, 1024], F32, tag="ps", name=f"psc{_i}")
                            for _i in range(2)
                        ]
                        # interleave subs so the two heads' 64-row groups
                        # run concurrently in the array
                        for qh in range(2):
                            for sub in range(2):
                                p0 = 64 * sub
                                nc.tensor.matmul(
                                    pscore[sub][:, qh * 512 : qh * 512 + 512],
                                    kT[hp][p0 : p0 + 64, ksl],
                                    qT[hp][p0 : p0 + 64, q0 + qh * 512 : q0 + qh * 512 + 512],
                                    start=True, stop=True,
                                )
                        ets = [None, None]
                        for sub in range(2):
                            et = epool.tile([128, 1024], BF16, tag="exp", name=f"et{sub}")
                            nc.scalar.activation(
                                out=et[:, :], in_=pscore[sub][:, :],
                                func=mybir.ActivationFunctionType.Exp,
                                scale=float(scale),
                            )
                            ets[sub] = et
                        for sub in range(2):
                            h = hp * 2 + sub
                            for qh in range(2):
                                nc.tensor.matmul(
                                    pctx[sub][:, qh * 512 : qh * 512 + 512],
                                    v_s[:, kt, h * VP : h * VP + VP],
                                    ets[sub][:, qh * 512 : qh * 512 + 512],
                                    start=(kt == 0), stop=(kt == NT - 1),
                                )
                    for sub in range(2):
                        h = hp * 2 + sub
                        # move ctx+sums to SBUF promptly so PSUM recycles;
                        # normalization then runs off the critical path
                        craw = smpool.tile([65, 1024], F32, tag="craw", bufs=2)
                        nc.vector.tensor_copy(out=craw[:, :], in_=pctx[sub][0:65, :])
                        rc = smpool.tile([65, 1024], F32, tag="rc", bufs=2)
                        nc.vector.reciprocal(out=rc[64:65, :], in_=craw[64:65, :])
                        pbc = ps.tile([128, 1024], F32, tag="ps")
                        for qh in range(2):
                            nc.tensor.matmul(
                                pbc[0:64, qh * 512 : qh * 512 + 512],
                                ones_s[64:65, 0:64],
                                rc[64:65, qh * 512 : qh * 512 + 512],
                                start=True, stop=True,
                            )
                        nc.vector.tensor_tensor(
                            ctx[h][:, bsl], craw[0:64, :], pbc[0:64, :],
                            mybir.AluOpType.mult,
                        )

                # ---- phase C: proj for the 8 n-tiles of this block ----
                for nti in range(8):
                    nt = blk * 8 + nti
                    po = ps.tile([128, 1024], F32, tag="ps")
                    for ocol in range(2):
                        for h in range(NH):
                            nc.tensor.matmul(
                                po[:, ocol * 512 : ocol * 512 + 384],
                                ctx[h][:, nt * 128 : nt * 128 + 128],
                                pw_s[:, h, ocol * 384 : ocol * 384 + 384],
                                start=(h == 0), stop=(h == NH - 1),
                            )
                    ot = opool.tile([128, C], F32, tag="ot")
                    nc.vector.tensor_copy(
                        out=ot[:, :].rearrange("p (o f) -> p o f", o=2),
                        in_=po[:, :].rearrange("p (o f) -> p o f", o=2)[:, :, 0:384],
                    )
                    nc.sync.dma_start(
                        out=out_bounce[nt * 128 : nt * 128 + 128, :],
                        in_=ot[:, :],
                    )

            # ---- ReduceScatter over the batch pair ----
            nc.gpsimd.collective_compute(
                "ReduceScatter",
                mybir.AluOpType.add,
                replica_groups=[[0, 1], [2, 3], [4, 5], [6, 7]],
                ins=[out_bounce.opt()],
                outs=[rs_out.opt()],
            )
            nc.sync.dma_start(out=out_d[:, :], in_=rs_out[:])

    nc.compile()
    return nc


NCOL4 = N // 512  # 4 columns of 512 for the qkv phase


def _prep_inputs(x, qkv_w, proj_w):
    """Build per-core input maps (host-side slicing/transposes only)."""
    import ml_dtypes
    bf16 = ml_dtypes.bfloat16
    ones = np.ones((128, 64), dtype=np.float32)
    in_maps = []
    for c in range(8):
        b, g = c // 2, c % 2
        sl = slice(g * HD, g * HD + HD)
        pwT = np.ascontiguousarray(proj_w[:, sl].T).astype(bf16)
        in_maps.append({
            "xT": np.ascontiguousarray(x[b].T),
            "qwT": np.ascontiguousarray(qkv_w[0 * C + g * HD : 0 * C + g * HD + HD, :].T),
            "kwT": np.ascontiguousarray(qkv_w[1 * C + g * HD : 1 * C + g * HD + HD, :].T),
            "vwT": np.ascontiguousarray(qkv_w[2 * C + g * HD : 2 * C + g * HD + HD, :].T),
            "pwT": pwT,
            "ones": ones,
        })
    return in_maps


def kernel(x, qkv_w, proj_w):
    from concourse.bass_utils import run_bass_kernel_spmd

    x = np.asarray(x, dtype=np.float32)
    qkv_w = np.asarray(qkv_w, dtype=np.float32)
    proj_w = np.asarray(proj_w, dtype=np.float32)

    if "nc" not in _CACHE:
        _CACHE["nc"] = build_nc()
    nc = _CACHE["nc"]

    in_maps = _prep_inputs(x, qkv_w, proj_w)
    res = run_bass_kernel_spmd(nc, in_maps, core_ids=list(range(8)))

    out = np.empty((B, N, C), dtype=np.float32)
    for b in range(B):
        out[b, : N // 2] = res.results[2 * b]["out"]
        out[b, N // 2 :] = res.results[2 * b + 1]["out"]
    return out


# revision 16
# speedup vs baseline: 1.2947x; 1.1079x over previous
"""Trainium2 Bass kernel: multi-head attention (B=4, N=2048, C=768, H=12, D=64).

Sharding over 8 NeuronCores: core c -> (batch b = c//2, head-group g = c%2).
Each head-group is 6 heads (384 channels). Per core:
  - qkv projection for its 6 heads (f32r matmuls, full-rate fp32)
  - per-head attention with transposed score layout scoreT[k, q]:
      * no max-subtraction (scores ~ N(0,1), exp is safe in fp32)
      * softmax denominator comes free from a ones-column appended to V
  - output projection against the head-group's slice of proj_w -> partial out
  - chunked ReduceScatter(add) over the pair sharing a batch; host reassembles.

All device tensors are pre-transposed on the host so the kernel needs no
on-chip transposes:
  xT   [768, 2048]  = x[b].T
  qwT/kwT/vwT [768, 384] = per-group qkv weight slices, transposed
  pwT  [384, 768]   = proj_w[:, group_cols].T

Perf notes (v4): q/k stored bf16 (fast background weight loads); V per-head
groups padded to 128 columns ([v | ones | zeros]) for FWL; exp in 1024-wide
blocks to amortize ScalarE per-op overhead; one head at a time through the
k loop so PSUM has 3 score slots and the PE never waits on the softmax;
xT streamed per 512-wide column block; normalization off the critical path.
"""

import sys

import numpy as np

B, N, C = 4, 2048, 768
H, D = 12, 64
NH = 6            # heads per core
HD = NH * D       # 384 channels per core
CT = C // 128     # 6 contraction tiles
NT = N // 128     # 16 k tiles of 128
NBLK = N // 1024  # 2 q blocks of 1024
NCOL4 = N // 512  # 4 columns of 512 for the qkv phase
HDT = HD // 128   # 3 head-dim tiles (2 heads each)
VP = 128          # padded v group: [v(64) | ones | zeros(63)]

_CACHE = {}


def build_nc():
    import concourse.bass as bass
    import concourse.mybir as mybir
    import concourse.tile as tile
    from concourse import bacc

    F32 = mybir.dt.float32
    F32R = mybir.dt.float32r
    BF16 = mybir.dt.bfloat16

    nc = bacc.Bacc("TRN2", target_bir_lowering=False, debug=False, num_devices=8)

    xT_d = nc.declare_dram_parameter("xT", [C, N], F32R, isOutput=False)
    qwT_d = nc.declare_dram_parameter("qwT", [C, HD], F32R, isOutput=False)
    kwT_d = nc.declare_dram_parameter("kwT", [C, HD], F32R, isOutput=False)
    vwT_d = nc.declare_dram_parameter("vwT", [C, HD], F32R, isOutput=False)
    pwT_d = nc.declare_dram_parameter("pwT", [HD, C], BF16, isOutput=False)
    ones_d = nc.declare_dram_parameter("ones", [128, 64], F32, isOutput=False)
    out_d = nc.declare_dram_parameter("out", [N // 2, C], F32, isOutput=True)

    scale = 1.0 / np.sqrt(D)

    with tile.TileContext(nc) as tc:
        with (
            tc.tile_pool(name="w", bufs=1) as wpool,
            tc.tile_pool(name="xtp", bufs=2) as xtp,
            tc.tile_pool(name="big", bufs=1) as big,
            tc.tile_pool(name="qk", bufs=1) as qkpool,
            tc.tile_pool(name="exp", bufs=8) as epool,
            tc.tile_pool(name="sm", bufs=3) as smpool,
            tc.tile_pool(name="outs", bufs=4) as opool,
            tc.tile_pool(name="ps", bufs=4, space="PSUM") as ps,
            tc.tile_pool(name="dram", bufs=1, space="DRAM") as dram,
        ):
            # ---- load weights & constants ----
            qw_s = wpool.tile([128, CT, HD], F32R, tag="qw")
            kw_s = wpool.tile([128, CT, HD], F32R, tag="kw")
            vw_s = wpool.tile([128, CT, HD], F32R, tag="vw")
            pw_s = wpool.tile([64, NH, C], BF16, tag="pw")
            ones_s = wpool.tile([128, 64], F32, tag="ones")
            nc.sync.dma_start(out=qw_s[:], in_=qwT_d.rearrange("(t p) f -> p t f", p=128))
            nc.sync.dma_start(out=kw_s[:], in_=kwT_d.rearrange("(t p) f -> p t f", p=128))
            nc.sync.dma_start(out=vw_s[:], in_=vwT_d.rearrange("(t p) f -> p t f", p=128))
            nc.sync.dma_start(out=pw_s[:], in_=pwT_d.rearrange("(h p) f -> p h f", p=64))
            nc.sync.dma_start(out=ones_s[:], in_=ones_d[:, :])

            # persistent SBUF tensors
            qT = [qkpool.tile([128, N], BF16, tag=f"qT{t}", name=f"qT{t}") for t in range(HDT)]
            kT = [qkpool.tile([128, N], BF16, tag=f"kT{t}", name=f"kT{t}") for t in range(HDT)]
            v_s = big.tile([128, NT, NH * VP], BF16, tag="v")
            ctx = [qkpool.tile([64, N], BF16, tag=f"ctx{h}", name=f"ctx{h}") for h in range(NH)]

            # v group layout per head: [v(64) | ones(1) | zeros(63)]
            v4 = v_s.rearrange("p n (h g) -> p n h g", h=NH)
            nc.vector.memset(v4[:, :, :, D:], 0.0)
            nc.vector.memset(v4[:, :, :, D : D + 1], 1.0)

            xT_r = xT_d.rearrange("(t p) n -> p t n", p=128)

            # ---- phase A: qkv projections, xT streamed per 512-col block ----
            for ncol in range(NCOL4):
                nsl = slice(ncol * 512, ncol * 512 + 512)
                xt_t = xtp.tile([128, CT, 512], F32R, tag="xt", name="xt_t")
                nc.sync.dma_start(out=xt_t[:], in_=xT_r[:, :, nsl])
                for hdt in range(HDT):
                    pq = ps.tile([128, 1024], F32, tag="ps")
                    for ct in range(CT):
                        nc.tensor.matmul(
                            pq[:, 0:512],
                            qw_s[:, ct, hdt * 128 : hdt * 128 + 128],
                            xt_t[:, ct, :],
                            start=(ct == 0), stop=(ct == CT - 1),
                        )
                    for ct in range(CT):
                        nc.tensor.matmul(
                            pq[:, 512:1024],
                            kw_s[:, ct, hdt * 128 : hdt * 128 + 128],
                            xt_t[:, ct, :],
                            start=(ct == 0), stop=(ct == CT - 1),
                        )
                    nc.vector.tensor_copy(out=qT[hdt][:, nsl], in_=pq[:, 0:512])
                    nc.vector.tensor_copy(out=kT[hdt][:, nsl], in_=pq[:, 512:1024])
                for nti in range(4):
                    nt = ncol * 4 + nti
                    pv = ps.tile([128, 1024], F32, tag="ps")
                    for ct in range(CT):
                        nc.tensor.matmul(
                            pv[:, :HD],
                            xt_t[:, ct, nti * 128 : nti * 128 + 128],
                            vw_s[:, ct, :],
                            start=(ct == 0), stop=(ct == CT - 1),
                        )
                    nc.vector.tensor_copy(
                        out=v4[:, nt, :, 0:D],
                        in_=pv[:, :HD].rearrange("p (h d) -> p h d", h=NH),
                    )

            # ---- phase B: attention ----
            # Both heads of a pair share one PSUM tile per 512-wide q column:
            # sub0 in cols 0:512, sub1 in cols 512:1024. Their score matmuls
            # use distinct 64-row array groups (concurrent), one 1024-wide exp
            # covers both, and 3 score slots give the PE lookahead.
            out_bounce = dram.tile([N, C], F32)
            rs_out = dram.tile([N // 2, C], F32)

            for qcol in range(NCOL4):
                qsl = slice(qcol * 512, qcol * 512 + 512)
                for hp in range(HDT):
                    pctx = ps.tile([128, 1024], F32, tag="ps", name="pctx")
                    for kt in range(NT):
                        ksl = slice(kt * 128, kt * 128 + 128)
                        psc = ps.tile([128, 1024], F32, tag="ps", name="psc")
                        for sub in range(2):
                            p0 = 64 * sub
                            nc.tensor.matmul(
                                psc[:, sub * 512 : sub * 512 + 512],
                                kT[hp][p0 : p0 + 64, ksl],
                                qT[hp][p0 : p0 + 64, qsl],
                                start=True, stop=True,
                            )
                        et = epool.tile([128, 1024], BF16, tag="exp", name="et")
                        nc.scalar.activation(
                            out=et[:, :], in_=psc[:, :],
                            func=mybir.ActivationFunctionType.Exp,
                            scale=float(scale),
                        )
                        for sub in range(2):
                            h = hp * 2 + sub
                            nc.tensor.matmul(
                                pctx[:, sub * 512 : sub * 512 + 512],
                                v_s[:, kt, h * VP : h * VP + VP],
                                et[:, sub * 512 : sub * 512 + 512],
                                start=(kt == 0), stop=(kt == NT - 1),
                            )
                    # move ctx+sums to SBUF promptly so PSUM recycles;
                    # normalization then runs off the critical path
                    craw = smpool.tile([65, 1024], F32, tag="craw", bufs=3)
                    nc.vector.tensor_copy(out=craw[:, :], in_=pctx[0:65, :])
                    rc = smpool.tile([65, 1024], F32, tag="rc", bufs=3)
                    nc.vector.reciprocal(out=rc[64:65, :], in_=craw[64:65, :])
                    pbc = ps.tile([128, 1024], F32, tag="ps", name="pbc")
                    for sub in range(2):
                        nc.tensor.matmul(
                            pbc[0:64, sub * 512 : sub * 512 + 512],
                            ones_s[64:65, 0:64],
                            rc[64:65, sub * 512 : sub * 512 + 512],
                            start=True, stop=True,
                        )
                    for sub in range(2):
                        h = hp * 2 + sub
                        nc.vector.tensor_tensor(
                            ctx[h][:, qsl],
                            craw[0:64, sub * 512 : sub * 512 + 512],
                            pbc[0:64, sub * 512 : sub * 512 + 512],
                            mybir.AluOpType.mult,
                        )

                # ---- phase C: proj for the 4 n-tiles of this q column ----
                for nti in range(4):
                    nt = qcol * 4 + nti
                    po = ps.tile([128, 1024], F32, tag="ps", name="po")
                    for ocol in range(2):
                        for h in range(NH):
                            nc.tensor.matmul(
                                po[:, ocol * 512 : ocol * 512 + 384],
                                ctx[h][:, nt * 128 : nt * 128 + 128],
                                pw_s[:, h, ocol * 384 : ocol * 384 + 384],
                                start=(h == 0), stop=(h == NH - 1),
                            )
                    ot = opool.tile([128, C], F32, tag="ot")
                    nc.vector.tensor_copy(
                        out=ot[:, :].rearrange("p (o f) -> p o f", o=2),
                        in_=po[:, :].rearrange("p (o f) -> p o f", o=2)[:, :, 0:384],
                    )
                    nc.sync.dma_start(
                        out=out_bounce[nt * 128 : nt * 128 + 128, :],
                        in_=ot[:, :],
                    )


            nc.gpsimd.collective_compute(
                "ReduceScatter",
                mybir.AluOpType.add,
                replica_groups=[[0, 1], [2, 3], [4, 5], [6, 7]],
                ins=[out_bounce.opt()],
                outs=[rs_out.opt()],
            )
            nc.sync.dma_start(out=out_d[:, :], in_=rs_out[:])

    nc.compile()
    return nc


def _prep_inputs(x, qkv_w, proj_w):
    """Build per-core input maps (host-side slicing/transposes only)."""
    import ml_dtypes
    bf16 = ml_dtypes.bfloat16
    ones = np.ones((128, 64), dtype=np.float32)
    in_maps = []
    for c in range(8):
        b, g = c // 2, c % 2
        sl = slice(g * HD, g * HD + HD)
        pwT = np.ascontiguousarray(proj_w[:, sl].T).astype(bf16)
        in_maps.append({
            "xT": np.ascontiguousarray(x[b].T),
            "qwT": np.ascontiguousarray(qkv_w[0 * C + g * HD : 0 * C + g * HD + HD, :].T),
            "kwT": np.ascontiguousarray(qkv_w[1 * C + g * HD : 1 * C + g * HD + HD, :].T),
            "vwT": np.ascontiguousarray(qkv_w[2 * C + g * HD : 2 * C + g * HD + HD, :].T),
            "pwT": pwT,
            "ones": ones,
        })
    return in_maps


def _kernel_impl(x, qkv_w, proj_w):
    from concourse.bass_utils import run_bass_kernel_spmd

    x = np.asarray(x, dtype=np.float32)
    qkv_w = np.asarray(qkv_w, dtype=np.float32)
    proj_w = np.asarray(proj_w, dtype=np.float32)

    if "nc" not in _CACHE:
        _CACHE["nc"] = build_nc()
    nc = _CACHE["nc"]

    in_maps = _prep_inputs(x, qkv_w, proj_w)
    res = run_bass_kernel_spmd(nc, in_maps, core_ids=list(range(8)))

    return _assemble(res.results)


def kernel(x, qkv_w, proj_w):
    """Run the device kernel in a child process with timeout + retries.

    The TRN terminal occasionally drops or hangs a run; a fresh process
    retry recovers. Falls back to an in-process run as a last resort.
    """
    import os
    import subprocess
    import sys
    import tempfile
    import time

    x = np.asarray(x, dtype=np.float32)
    qkv_w = np.asarray(qkv_w, dtype=np.float32)
    proj_w = np.asarray(proj_w, dtype=np.float32)

    tmpd = tempfile.mkdtemp(prefix="trnkern_")
    inp = os.path.join(tmpd, "in.npz")
    outp = os.path.join(tmpd, "out.npz")
    np.savez(inp, x=x, qkv_w=qkv_w, proj_w=proj_w)
    for attempt in range(3):
        try:
            r = subprocess.run(
                [sys.executable, os.path.abspath(__file__), inp, outp],
                timeout=900, capture_output=True,
            )
            if r.returncode == 0 and os.path.exists(outp):
                return np.load(outp)["out"]
            sys.stderr.write(
                f"kernel subprocess attempt {attempt} rc={r.returncode}\n"
                + r.stderr.decode(errors="replace")[-2000:] + "\n"
            )
        except subprocess.TimeoutExpired:
            sys.stderr.write(f"kernel subprocess attempt {attempt} timed out\n")
        time.sleep(5)
    return _kernel_impl(x, qkv_w, proj_w)


if __name__ == "__main__":
    _inp, _outp = sys.argv[1], sys.argv[2]
    import sys as _sys

    _d = np.load(_inp)
    _out = _kernel_impl(_d["x"], _d["qkv_w"], _d["proj_w"])
    np.savez(_outp, out=_out)


def _assemble(results):
    out = np.empty((B, N, C), dtype=np.float32)
    for b in range(B):
        out[b, : N // 2] = results[2 * b]["out"]
        out[b, N // 2 :] = results[2 * b + 1]["out"]
    return out


# revision 17
# speedup vs baseline: 1.4038x; 1.0842x over previous
"""Trainium2 Bass kernel: multi-head attention (B=4, N=2048, C=768, H=12, D=64).

Sharding over 8 NeuronCores: core c -> (batch b = c//2, head-group g = c%2).
Each head-group is 6 heads (384 channels). Per core:
  - qkv projection for its 6 heads (f32r matmuls, full-rate fp32)
  - per-head attention with transposed score layout scoreT[k, q]:
      * no max-subtraction (scores ~ N(0,1), exp is safe in fp32)
      * softmax denominator comes free from a ones-column appended to V
  - output projection against the head-group's slice of proj_w -> partial out
  - chunked ReduceScatter(add) over the pair sharing a batch; host reassembles.

All device tensors are pre-transposed on the host so the kernel needs no
on-chip transposes:
  xT   [768, 2048]  = x[b].T
  qwT/kwT/vwT [768, 384] = per-group qkv weight slices, transposed
  pwT  [384, 768]   = proj_w[:, group_cols].T

Perf notes (v4): q/k stored bf16 (fast background weight loads); V per-head
groups padded to 128 columns ([v | ones | zeros]) for FWL; exp in 1024-wide
blocks to amortize ScalarE per-op overhead; one head at a time through the
k loop so PSUM has 3 score slots and the PE never waits on the softmax;
xT streamed per 512-wide column block; normalization off the critical path.
"""

import sys

import numpy as np

B, N, C = 4, 2048, 768
H, D = 12, 64
NH = 6            # heads per core
HD = NH * D       # 384 channels per core
CT = C // 128     # 6 contraction tiles
NT = N // 128     # 16 k tiles of 128
NBLK = N // 1024  # 2 q blocks of 1024
NCOL4 = N // 512  # 4 columns of 512 for the qkv phase
HDT = HD // 128   # 3 head-dim tiles (2 heads each)
VP = 128          # padded v group: [v(64) | ones | zeros(63)]

_CACHE = {}


def build_nc():
    import concourse.bass as bass
    import concourse.mybir as mybir
    import concourse.tile as tile
    from concourse import bacc

    F32 = mybir.dt.float32
    F32R = mybir.dt.float32r
    BF16 = mybir.dt.bfloat16

    nc = bacc.Bacc("TRN2", target_bir_lowering=False, debug=False, num_devices=8)

    xT_d = nc.declare_dram_parameter("xT", [C, N], F32R, isOutput=False)
    qwT_d = nc.declare_dram_parameter("qwT", [C, HD], F32R, isOutput=False)
    kwT_d = nc.declare_dram_parameter("kwT", [C, HD], F32R, isOutput=False)
    vwT_d = nc.declare_dram_parameter("vwT", [C, HD], F32R, isOutput=False)
    pwT_d = nc.declare_dram_parameter("pwT", [HD, C], BF16, isOutput=False)
    ones_d = nc.declare_dram_parameter("ones", [128, 64], F32, isOutput=False)
    out_d = nc.declare_dram_parameter("out", [N // 2, C], F32, isOutput=True)

    scale = 1.0 / np.sqrt(D)

    with tile.TileContext(nc) as tc:
        with (
            tc.tile_pool(name="w", bufs=1) as wpool,
            tc.tile_pool(name="xtp", bufs=2) as xtp,
            tc.tile_pool(name="big", bufs=1) as big,
            tc.tile_pool(name="qk", bufs=1) as qkpool,
            tc.tile_pool(name="exp", bufs=8) as epool,
            tc.tile_pool(name="sm", bufs=3) as smpool,
            tc.tile_pool(name="outs", bufs=4) as opool,
            tc.tile_pool(name="ps", bufs=4, space="PSUM") as ps,
            tc.tile_pool(name="dram", bufs=1, space="DRAM") as dram,
        ):
            # ---- load weights & constants ----
            qw_s = wpool.tile([128, CT, HD], F32R, tag="qw")
            kw_s = wpool.tile([128, CT, HD], F32R, tag="kw")
            vw_s = wpool.tile([128, CT, HD], F32R, tag="vw")
            pw_s = wpool.tile([64, NH, C], BF16, tag="pw")
            ones_s = wpool.tile([128, 64], F32, tag="ones")
            nc.sync.dma_start(out=qw_s[:], in_=qwT_d.rearrange("(t p) f -> p t f", p=128))
            nc.sync.dma_start(out=kw_s[:], in_=kwT_d.rearrange("(t p) f -> p t f", p=128))
            nc.sync.dma_start(out=vw_s[:], in_=vwT_d.rearrange("(t p) f -> p t f", p=128))
            nc.sync.dma_start(out=pw_s[:], in_=pwT_d.rearrange("(h p) f -> p h f", p=64))
            nc.sync.dma_start(out=ones_s[:], in_=ones_d[:, :])

            # persistent SBUF tensors
            qT = [qkpool.tile([128, N], BF16, tag=f"qT{t}", name=f"qT{t}") for t in range(HDT)]
            kT = [qkpool.tile([128, N], BF16, tag=f"kT{t}", name=f"kT{t}") for t in range(HDT)]
            v_s = big.tile([128, NT, NH * VP], BF16, tag="v")
            ctx = [qkpool.tile([64, N], BF16, tag=f"ctx{h}", name=f"ctx{h}") for h in range(NH)]

            # v group layout per head: [v(64) | ones(1) | zeros(63)]
            v4 = v_s.rearrange("p n (h g) -> p n h g", h=NH)
            nc.vector.memset(v4[:, :, :, D:], 0.0)
            nc.vector.memset(v4[:, :, :, D : D + 1], 1.0)

            xT_r = xT_d.rearrange("(t p) n -> p t n", p=128)

            # ---- phase A: qkv projections, xT streamed per 512-col block ----
            for ncol in range(NCOL4):
                nsl = slice(ncol * 512, ncol * 512 + 512)
                xt_t = xtp.tile([128, CT, 512], F32R, tag="xt", name="xt_t")
                nc.sync.dma_start(out=xt_t[:], in_=xT_r[:, :, nsl])
                for hdt in range(HDT):
                    pq = ps.tile([128, 1024], F32, tag="ps")
                    for ct in range(CT):
                        nc.tensor.matmul(
                            pq[:, 0:512],
                            qw_s[:, ct, hdt * 128 : hdt * 128 + 128],
                            xt_t[:, ct, :],
                            start=(ct == 0), stop=(ct == CT - 1),
                        )
                    for ct in range(CT):
                        nc.tensor.matmul(
                            pq[:, 512:1024],
                            kw_s[:, ct, hdt * 128 : hdt * 128 + 128],
                            xt_t[:, ct, :],
                            start=(ct == 0), stop=(ct == CT - 1),
                        )
                    nc.vector.tensor_copy(out=qT[hdt][:, nsl], in_=pq[:, 0:512])
                    nc.vector.tensor_copy(out=kT[hdt][:, nsl], in_=pq[:, 512:1024])
                for nti in range(4):
                    nt = ncol * 4 + nti
                    pv = ps.tile([128, 1024], F32, tag="ps")
                    for ct in range(CT):
                        nc.tensor.matmul(
                            pv[:, :HD],
                            xt_t[:, ct, nti * 128 : nti * 128 + 128],
                            vw_s[:, ct, :],
                            start=(ct == 0), stop=(ct == CT - 1),
                        )
                    nc.vector.tensor_copy(
                        out=v4[:, nt, :, 0:D],
                        in_=pv[:, :HD].rearrange("p (h d) -> p h d", h=NH),
                    )

            # ---- phase B: attention ----
            # Both heads of a pair share one PSUM tile per 512-wide q column:
            # sub0 in cols 0:512, sub1 in cols 512:1024. Their score matmuls
            # use distinct 64-row array groups (concurrent), one 1024-wide exp
            # covers both, and 3 score slots give the PE lookahead.
            out_bounce = dram.tile([N, C], F32)
            rs_out = dram.tile([N // 2, C], F32)

            for qcol in range(NCOL4):
                qsl = slice(qcol * 512, qcol * 512 + 512)
                for hp in range(HDT):
                    pctx = ps.tile([128, 1024], F32, tag="ps", name="pctx")
                    for kt in range(NT):
                        ksl = slice(kt * 128, kt * 128 + 128)
                        psc = ps.tile([128, 1024], F32, tag="ps", name="psc")
                        for sub in range(2):
                            p0 = 64 * sub
                            nc.tensor.matmul(
                                psc[:, sub * 512 : sub * 512 + 512],
                                kT[hp][p0 : p0 + 64, ksl],
                                qT[hp][p0 : p0 + 64, qsl],
                                start=True, stop=True,
                            )
                        et = epool.tile([128, 1024], BF16, tag="exp", name="et")
                        nc.scalar.activation(
                            out=et[:, :], in_=psc[:, :],
                            func=mybir.ActivationFunctionType.Exp,
                            scale=float(scale),
                        )
                        for sub in range(2):
                            h = hp * 2 + sub
                            nc.tensor.matmul(
                                pctx[:, sub * 512 : sub * 512 + 512],
                                v_s[:, kt, h * VP : h * VP + VP],
                                et[:, sub * 512 : sub * 512 + 512],
                                start=(kt == 0), stop=(kt == NT - 1),
                            )
                    # move ctx+sums to SBUF promptly so PSUM recycles.
                    # Broadcast the sums (not their reciprocals) via the
                    # outer product so the PE only waits on the craw copy;
                    # the reciprocal runs on the broadcast, purely on DVE,
                    # off the PE critical path.
                    craw = smpool.tile([65, 1024], F32, tag="craw", bufs=3)
                    nc.vector.tensor_copy(out=craw[:, :], in_=pctx[0:65, :])
                    pbs = ps.tile([128, 1024], F32, tag="ps", name="pbs")
                    for sub in range(2):
                        nc.tensor.matmul(
                            pbs[0:64, sub * 512 : sub * 512 + 512],
                            ones_s[64:65, 0:64],
                            craw[64:65, sub * 512 : sub * 512 + 512],
                            start=True, stop=True,
                        )
                    bcr = smpool.tile([64, 1024], F32, tag="bcr", bufs=2)
                    nc.vector.reciprocal(out=bcr[:, :], in_=pbs[0:64, :])
                    for sub in range(2):
                        h = hp * 2 + sub
                        nc.vector.tensor_tensor(
                            ctx[h][:, qsl],
                            craw[0:64, sub * 512 : sub * 512 + 512],
                            bcr[:, sub * 512 : sub * 512 + 512],
                            mybir.AluOpType.mult,
                        )

                # ---- phase C: proj for the 4 n-tiles of this q column ----
                for nti in range(4):
                    nt = qcol * 4 + nti
                    po = ps.tile([128, 1024], F32, tag="ps", name="po")
                    for ocol in range(2):
                        for h in range(NH):
                            nc.tensor.matmul(
                                po[:, ocol * 512 : ocol * 512 + 384],
                                ctx[h][:, nt * 128 : nt * 128 + 128],
                                pw_s[:, h, ocol * 384 : ocol * 384 + 384],
                                start=(h == 0), stop=(h == NH - 1),
                            )
                    ot = opool.tile([128, C], F32, tag="ot")
                    nc.vector.tensor_copy(
                        out=ot[:, :].rearrange("p (o f) -> p o f", o=2),
                        in_=po[:, :].rearrange("p (o f) -> p o f", o=2)[:, :, 0:384],
                    )
                    nc.sync.dma_start(
                        out=out_bounce[nt * 128 : nt * 128 + 128, :],
                        in_=ot[:, :],
                    )


            nc.gpsimd.collective_compute(
                "ReduceScatter",
                mybir.AluOpType.add,
                replica_groups=[[0, 1], [2, 3], [4, 5], [6, 7]],
                ins=[out_bounce.opt()],
                outs=[rs_out.opt()],
            )
            nc.sync.dma_start(out=out_d[:, :], in_=rs_out[:])

    nc.compile()
    return nc


def _prep_inputs(x, qkv_w, proj_w):
    """Build per-core input maps (host-side slicing/transposes only)."""
    import ml_dtypes
    bf16 = ml_dtypes.bfloat16
    ones = np.ones((128, 64), dtype=np.float32)
    in_maps = []
    for c in range(8):
        b, g = c // 2, c % 2
        sl = slice(g * HD, g * HD + HD)
        pwT = np.ascontiguousarray(proj_w[:, sl].T).astype(bf16)
        in_maps.append({
            "xT": np.ascontiguousarray(x[b].T),
            "qwT": np.ascontiguousarray(qkv_w[0 * C + g * HD : 0 * C + g * HD + HD, :].T),
            "kwT": np.ascontiguousarray(qkv_w[1 * C + g * HD : 1 * C + g * HD + HD, :].T),
            "vwT": np.ascontiguousarray(qkv_w[2 * C + g * HD : 2 * C + g * HD + HD, :].T),
            "pwT": pwT,
            "ones": ones,
        })
    return in_maps


def _kernel_impl(x, qkv_w, proj_w):
    from concourse.bass_utils import run_bass_kernel_spmd

    x = np.asarray(x, dtype=np.float32)
    qkv_w = np.asarray(qkv_w, dtype=np.float32)
    proj_w = np.asarray(proj_w, dtype=np.float32)

    if "nc" not in _CACHE:
        _CACHE["nc"] = build_nc()
    nc = _CACHE["nc"]

    in_maps = _prep_inputs(x, qkv_w, proj_w)
    res = run_bass_kernel_spmd(nc, in_maps, core_ids=list(range(8)))

    return _assemble(res.results)


def kernel(x, qkv_w, proj_w):
    """Run the device kernel in a child process with timeout + retries.

    The TRN terminal occasionally drops or hangs a run; a fresh process
    retry recovers. Falls back to an in-process run as a last resort.
    """
    import os
    import subprocess
    import sys
    import tempfile
    import time

    x = np.asarray(x, dtype=np.float32)
    qkv_w = np.asarray(qkv_w, dtype=np.float32)
    proj_w = np.asarray(proj_w, dtype=np.float32)

    tmpd = tempfile.mkdtemp(prefix="trnkern_")
    inp = os.path.join(tmpd, "in.npz")
    outp = os.path.join(tmpd, "out.npz")
    np.savez(inp, x=x, qkv_w=qkv_w, proj_w=proj_w)
    for attempt in range(3):
        try:
            r = subprocess.run(
                [sys.executable, os.path.abspath(__file__), inp, outp],
                timeout=900, capture_output=True,
            )
            if r.returncode == 0 and os.path.exists(outp):
                return np.load(outp)["out"]
            sys.stderr.write(
                f"kernel subprocess attempt {attempt} rc={r.returncode}\n"
                + r.stderr.decode(errors="replace")[-2000:] + "\n"
            )
        except subprocess.TimeoutExpired:
            sys.stderr.write(f"kernel subprocess attempt {attempt} timed out\n")
        time.sleep(5)
    return _kernel_impl(x, qkv_w, proj_w)


if __name__ == "__main__":
    _inp, _outp = sys.argv[1], sys.argv[2]
    import sys as _sys

    _d = np.load(_inp)
    _out = _kernel_impl(_d["x"], _d["qkv_w"], _d["proj_w"])
    np.savez(_outp, out=_out)


def _assemble(results):
    out = np.empty((B, N, C), dtype=np.float32)
    for b in range(B):
        out[b, : N // 2] = results[2 * b]["out"]
        out[b, N // 2 :] = results[2 * b + 1]["out"]
    return out


# revision 19
# speedup vs baseline: 1.5201x; 1.0829x over previous
"""Trainium2 Bass kernel: multi-head attention (B=4, N=2048, C=768, H=12, D=64).

Sharding over 8 NeuronCores: core c -> (batch b = c//2, head-group g = c%2).
Each head-group is 6 heads (384 channels). Per core:
  - qkv projection for its 6 heads (f32r matmuls, full-rate fp32)
  - per-head attention with transposed score layout scoreT[k, q]:
      * no max-subtraction (scores ~ N(0,1), exp is safe in fp32)
      * softmax denominator comes free from a ones-column appended to V
  - output projection against the head-group's slice of proj_w -> partial out
  - chunked ReduceScatter(add) over the pair sharing a batch; host reassembles.

All device tensors are pre-transposed on the host so the kernel needs no
on-chip transposes:
  xT   [768, 2048]  = x[b].T
  qwT/kwT/vwT [768, 384] = per-group qkv weight slices, transposed
  pwT  [384, 768]   = proj_w[:, group_cols].T

Perf notes (v4): q/k stored bf16 (fast background weight loads); V per-head
groups padded to 128 columns ([v | ones | zeros]) for FWL; exp in 1024-wide
blocks to amortize ScalarE per-op overhead; one head at a time through the
k loop so PSUM has 3 score slots and the PE never waits on the softmax;
xT streamed per 512-wide column block; normalization off the critical path.
"""

import sys

import numpy as np

B, N, C = 4, 2048, 768
H, D = 12, 64
NH = 6            # heads per core
HD = NH * D       # 384 channels per core
CT = C // 128     # 6 contraction tiles
NT = N // 128     # 16 k tiles of 128
NBLK = N // 1024  # 2 q blocks of 1024
NCOL4 = N // 512  # 4 columns of 512 for the qkv phase
HDT = HD // 128   # 3 head-dim tiles (2 heads each)
VP = 128          # padded v group: [v(64) | ones | zeros(63)]

_CACHE = {}


def build_nc():
    import concourse.bass as bass
    import concourse.mybir as mybir
    import concourse.tile as tile
    from concourse import bacc

    F32 = mybir.dt.float32
    F32R = mybir.dt.float32r
    BF16 = mybir.dt.bfloat16

    nc = bacc.Bacc("TRN2", target_bir_lowering=False, debug=False, num_devices=8)

    xT_d = nc.declare_dram_parameter("xT", [C, N], F32R, isOutput=False)
    qwT_d = nc.declare_dram_parameter("qwT", [C, HD], F32R, isOutput=False)
    kwT_d = nc.declare_dram_parameter("kwT", [C, HD], F32R, isOutput=False)
    vwT_d = nc.declare_dram_parameter("vwT", [C, HD], F32R, isOutput=False)
    pwT_d = nc.declare_dram_parameter("pwT", [HD, C], BF16, isOutput=False)
    ones_d = nc.declare_dram_parameter("ones", [128, 64], F32, isOutput=False)
    out_d = nc.declare_dram_parameter("out", [N // 2, C], F32, isOutput=True)

    scale = 1.0 / np.sqrt(D)

    with tile.TileContext(nc) as tc:
        with (
            tc.tile_pool(name="w", bufs=1) as wpool,
            tc.tile_pool(name="xtp", bufs=2) as xtp,
            tc.tile_pool(name="big", bufs=1) as big,
            tc.tile_pool(name="qk", bufs=1) as qkpool,
            tc.tile_pool(name="exp", bufs=8) as epool,
            tc.tile_pool(name="sm", bufs=3) as smpool,
            tc.tile_pool(name="outs", bufs=4) as opool,
            tc.tile_pool(name="ps", bufs=4, space="PSUM") as ps,
            tc.tile_pool(name="dram", bufs=1, space="DRAM") as dram,
        ):
            # ---- load weights & constants ----
            qw_s = wpool.tile([128, CT, HD], F32R, tag="qw")
            kw_s = wpool.tile([128, CT, HD], F32R, tag="kw")
            vw_s = wpool.tile([128, CT, HD], F32R, tag="vw")
            pw_s = wpool.tile([64, NH, C], BF16, tag="pw")
            ones_s = wpool.tile([128, 64], F32, tag="ones")
            nc.sync.dma_start(out=qw_s[:], in_=qwT_d.rearrange("(t p) f -> p t f", p=128))
            nc.sync.dma_start(out=kw_s[:], in_=kwT_d.rearrange("(t p) f -> p t f", p=128))
            nc.sync.dma_start(out=vw_s[:], in_=vwT_d.rearrange("(t p) f -> p t f", p=128))
            nc.sync.dma_start(out=pw_s[:], in_=pwT_d.rearrange("(h p) f -> p h f", p=64))
            nc.sync.dma_start(out=ones_s[:], in_=ones_d[:, :])

            # persistent SBUF tensors
            qT = [qkpool.tile([128, N], BF16, tag=f"qT{t}", name=f"qT{t}") for t in range(HDT)]
            kT = [qkpool.tile([128, N], BF16, tag=f"kT{t}", name=f"kT{t}") for t in range(HDT)]
            v_s = big.tile([128, NT, NH * VP], BF16, tag="v")
            ctx = [qkpool.tile([64, N], BF16, tag=f"ctx{h}", name=f"ctx{h}") for h in range(NH)]

            # v group layout per head: [v(64) | ones(1) | zeros(63)]
            v4 = v_s.rearrange("p n (h g) -> p n h g", h=NH)
            nc.vector.memset(v4[:, :, :, D:], 0.0)
            nc.vector.memset(v4[:, :, :, D : D + 1], 1.0)

            xT_r = xT_d.rearrange("(t p) n -> p t n", p=128)

            # ---- phase A: qkv projections, xT streamed per 512-col block ----
            for ncol in range(NCOL4):
                nsl = slice(ncol * 512, ncol * 512 + 512)
                xt_t = xtp.tile([128, CT, 512], F32R, tag="xt", name="xt_t")
                nc.sync.dma_start(out=xt_t[:], in_=xT_r[:, :, nsl])
                for hdt in range(HDT):
                    pq = ps.tile([128, 1024], F32, tag="ps")
                    for ct in range(CT):
                        nc.tensor.matmul(
                            pq[:, 0:512],
                            qw_s[:, ct, hdt * 128 : hdt * 128 + 128],
                            xt_t[:, ct, :],
                            start=(ct == 0), stop=(ct == CT - 1),
                        )
                    for ct in range(CT):
                        nc.tensor.matmul(
                            pq[:, 512:1024],
                            kw_s[:, ct, hdt * 128 : hdt * 128 + 128],
                            xt_t[:, ct, :],
                            start=(ct == 0), stop=(ct == CT - 1),
                        )
                    nc.vector.tensor_copy(out=qT[hdt][:, nsl], in_=pq[:, 0:512])
                    nc.vector.tensor_copy(out=kT[hdt][:, nsl], in_=pq[:, 512:1024])
                for nti in range(4):
                    nt = ncol * 4 + nti
                    pv = ps.tile([128, 1024], F32, tag="ps")
                    for ct in range(CT):
                        nc.tensor.matmul(
                            pv[:, :HD],
                            xt_t[:, ct, nti * 128 : nti * 128 + 128],
                            vw_s[:, ct, :],
                            start=(ct == 0), stop=(ct == CT - 1),
                        )
                    nc.vector.tensor_copy(
                        out=v4[:, nt, :, 0:D],
                        in_=pv[:, :HD].rearrange("p (h d) -> p h d", h=NH),
                    )

            # ---- phase B: attention ----
            # Both heads of a pair share one PSUM tile per 512-wide q column:
            # sub0 in cols 0:512, sub1 in cols 512:1024. Their score matmuls
            # use distinct 64-row array groups (concurrent), one 1024-wide exp
            # covers both, and 3 score slots give the PE lookahead.
            out_bounce = dram.tile([N, C], F32)
            rs_out = dram.tile([N // 2, C], F32)

            for qcol in range(NCOL4):
                qsl = slice(qcol * 512, qcol * 512 + 512)
                for hp in range(HDT):
                    pctx = ps.tile([128, 1024], F32, tag="ps", name="pctx")
                    for kt in range(NT):
                        ksl = slice(kt * 128, kt * 128 + 128)
                        psc = ps.tile([128, 1024], F32, tag="ps", name="psc")
                        for sub in range(2):
                            p0 = 64 * sub
                            nc.tensor.matmul(
                                psc[:, sub * 512 : sub * 512 + 512],
                                kT[hp][p0 : p0 + 64, ksl],
                                qT[hp][p0 : p0 + 64, qsl],
                                start=True, stop=True,
                            )
                        et = epool.tile([128, 1024], BF16, tag="exp", name="et")
                        nc.scalar.activation(
                            out=et[:, :], in_=psc[:, :],
                            func=mybir.ActivationFunctionType.Exp,
                            scale=float(scale),
                        )
                        for sub in range(2):
                            h = hp * 2 + sub
                            nc.tensor.matmul(
                                pctx[:, sub * 512 : sub * 512 + 512],
                                v_s[:, kt, h * VP : h * VP + VP],
                                et[:, sub * 512 : sub * 512 + 512],
                                start=(kt == 0), stop=(kt == NT - 1),
                            )
                    # move ctx+sums to SBUF promptly so PSUM recycles.
                    # Broadcast the sums (not their reciprocals) via the
                    # outer product so the PE only waits on the craw copy;
                    # the reciprocal runs on the broadcast, purely on DVE,
                    # off the PE critical path.
                    craw = smpool.tile([65, 1024], F32, tag="craw", bufs=3)
                    nc.vector.tensor_copy(out=craw[:, :], in_=pctx[0:65, :])
                    pbs = ps.tile([128, 1024], F32, tag="ps", name="pbs")
                    for sub in range(2):
                        nc.tensor.matmul(
                            pbs[0:64, sub * 512 : sub * 512 + 512],
                            ones_s[64:65, 0:64],
                            craw[64:65, sub * 512 : sub * 512 + 512],
                            start=True, stop=True,
                        )
                    bcs = smpool.tile([64, 1024], F32, tag="bcs", bufs=2)
                    nc.vector.tensor_copy(out=bcs[:, :], in_=pbs[0:64, :])
                    bcr = smpool.tile([64, 1024], F32, tag="bcr", bufs=2)
                    nc.vector.reciprocal(out=bcr[:, :], in_=bcs[:, :])
                    for sub in range(2):
                        h = hp * 2 + sub
                        nc.vector.tensor_tensor(
                            ctx[h][:, qsl],
                            craw[0:64, sub * 512 : sub * 512 + 512],
                            bcr[:, sub * 512 : sub * 512 + 512],
                            mybir.AluOpType.mult,
                        )

                # ---- phase C: proj for the 4 n-tiles of this q column ----
                for nti in range(4):
                    nt = qcol * 4 + nti
                    po = ps.tile([128, 1024], F32, tag="ps", name="po")
                    for ocol in range(2):
                        for h in range(NH):
                            nc.tensor.matmul(
                                po[:, ocol * 512 : ocol * 512 + 384],
                                ctx[h][:, nt * 128 : nt * 128 + 128],
                                pw_s[:, h, ocol * 384 : ocol * 384 + 384],
                                start=(h == 0), stop=(h == NH - 1),
                            )
                    ot = opool.tile([128, C], F32, tag="ot")
                    nc.vector.tensor_copy(
                        out=ot[:, :].rearrange("p (o f) -> p o f", o=2),
                        in_=po[:, :].rearrange("p (o f) -> p o f", o=2)[:, :, 0:384],
                    )
                    nc.sync.dma_start(
                        out=out_bounce[nt * 128 : nt * 128 + 128, :],
                        in_=ot[:, :],
                    )


            nc.gpsimd.collective_compute(
                "ReduceScatter",
                mybir.AluOpType.add,
                replica_groups=[[0, 1], [2, 3], [4, 5], [6, 7]],
                ins=[out_bounce.opt()],
                outs=[rs_out.opt()],
            )
            nc.sync.dma_start(out=out_d[:, :], in_=rs_out[:])

    nc.compile()
    return nc


def _prep_inputs(x, qkv_w, proj_w):
    """Build per-core input maps (host-side slicing/transposes only)."""
    import ml_dtypes
    bf16 = ml_dtypes.bfloat16
    ones = np.ones((128, 64), dtype=np.float32)
    in_maps = []
    for c in range(8):
        b, g = c // 2, c % 2
        sl = slice(g * HD, g * HD + HD)
        pwT = np.ascontiguousarray(proj_w[:, sl].T).astype(bf16)
        in_maps.append({
            "xT": np.ascontiguousarray(x[b].T),
            "qwT": np.ascontiguousarray(qkv_w[0 * C + g * HD : 0 * C + g * HD + HD, :].T),
            "kwT": np.ascontiguousarray(qkv_w[1 * C + g * HD : 1 * C + g * HD + HD, :].T),
            "vwT": np.ascontiguousarray(qkv_w[2 * C + g * HD : 2 * C + g * HD + HD, :].T),
            "pwT": pwT,
            "ones": ones,
        })
    return in_maps


def _kernel_impl(x, qkv_w, proj_w):
    from concourse.bass_utils import run_bass_kernel_spmd

    x = np.asarray(x, dtype=np.float32)
    qkv_w = np.asarray(qkv_w, dtype=np.float32)
    proj_w = np.asarray(proj_w, dtype=np.float32)

    if "nc" not in _CACHE:
        _CACHE["nc"] = build_nc()
    nc = _CACHE["nc"]

    in_maps = _prep_inputs(x, qkv_w, proj_w)
    res = run_bass_kernel_spmd(nc, in_maps, core_ids=list(range(8)))

    return _assemble(res.results)


def kernel(x, qkv_w, proj_w):
    """Run the device kernel in a child process with timeout + retries.

    The TRN terminal occasionally drops or hangs a run; a fresh process
    retry recovers. Falls back to an in-process run as a last resort.
    """
    import os
    import subprocess
    import sys
    import tempfile
    import time

    x = np.asarray(x, dtype=np.float32)
    qkv_w = np.asarray(qkv_w, dtype=np.float32)
    proj_w = np.asarray(proj_w, dtype=np.float32)

    tmpd = tempfile.mkdtemp(prefix="trnkern_")
    inp = os.path.join(tmpd, "in.npz")
    outp = os.path.join(tmpd, "out.npz")
    np.savez(inp, x=x, qkv_w=qkv_w, proj_w=proj_w)
    for attempt in range(3):
        try:
            r = subprocess.run(
                [sys.executable, os.path.abspath(__file__), inp, outp],
                timeout=900, capture_output=True,
            )
            if r.returncode == 0 and os.path.exists(outp):
                return np.load(outp)["out"]
            sys.stderr.write(
                f"kernel subprocess attempt {attempt} rc={r.returncode}\n"
                + r.stderr.decode(errors="replace")[-2000:] + "\n"
            )
        except subprocess.TimeoutExpired:
            sys.stderr.write(f"kernel subprocess attempt {attempt} timed out\n")
        time.sleep(5)
    return _kernel_impl(x, qkv_w, proj_w)


if __name__ == "__main__":
    _inp, _outp = sys.argv[1], sys.argv[2]
    import sys as _sys

    _d = np.load(_inp)
    _out = _kernel_impl(_d["x"], _d["qkv_w"], _d["proj_w"])
    np.savez(_outp, out=_out)


def _assemble(results):
    out = np.empty((B, N, C), dtype=np.float32)
    for b in range(B):
        out[b, : N // 2] = results[2 * b]["out"]
        out[b, N // 2 :] = results[2 * b + 1]["out"]
    return out
